# revision 83
# baseline (speedup 1.0000x reference)
"""Causal self-attention (B=4, T=2048, C=1024, H=16, D=64) on 8 Trainium2 cores.

Sharding: core c = (b, hg) with b = c // 2 (batch), hg = c % 2 (head-group of
8 heads = 512 of 1024 qkv columns). Each core computes q/k/v projections for
its (b, hg), causal attention for its 8 heads, and a partial output
projection y_hg @ Wp[hg]. Host sums the two head-group partials per batch and
adds the projection bias.

Per-core kernel (all matmuls bf16, PSUM accumulation + softmax in fp32):

  - qT/kT [col, t] via lhsT = weight chunk, rhs = xT chunk; v natural [t, col]
    via lhsT = xT chunk, rhs = Wv; v is stored in 65-wide groups per head with
    a ones-column.
  - scoresT chunks [s=128, t<=512] on PE, block-diagonal chunks packed
    compactly so one ScalarE exp instruction covers each chunk pair (no
    max-subtraction: logits are ~N(0,1); fp32 exp cannot overflow). Causal
    zeroing of diagonal chunks multiplies by a precomputed 0/1 mask (DVE).
  - AV runs TRANSPOSED for full PE-array utilization: lhsT = att chunk
    [s=128, t=128] (stationary), rhs = [v | ones] [s=128, 65] (moving),
    accumulating yz[t=128, 65] per (head, t-chunk) in PSUM; column 64 is the
    softmax denominator Z. One accumulation group per head (PSUM allows one
    open group per 2KB bank; first write per address overwrites). Rows are
    scaled by 1/Z into y natural [t, col] (bf16), then PE-transposed back to
    [col, t] for the output projection.

Scheduling: the attention stream is software-pipelined -- scores+exp of
chunk-pair i+1 are emitted BEFORE mask+AV of pair i, so ScalarE's exp always
overlaps PE work (the PE wait queue releases in order, so anything emitted
behind a parked AV cannot run early). Projection matmuls ride in two filler
queues consumed between pipeline stages: `gated` (q/k/v projections, whose
execution also gates attention heads via per-pair markers) and `reserve`
(output projections, saved for the exp-heaviest last quarter). Startup
streams x/Wv in interleaved chunks consumed chunk-major by four concurrent
v-unit accumulators; the final projection borrows the idle score-pool PSUM
banks so four output groups pipeline into the drain.
"""

import sys

if "/opt/trn_rl_repo" not in sys.path:
    sys.path.insert(0, "/opt/trn_rl_repo")

from contextlib import ExitStack

import numpy as np

import concourse.mybir as mybir
import concourse.tile as tile
from concourse import bacc, masks
from concourse.bass_utils import run_bass_kernel_spmd

F32 = mybir.dt.float32
BF16 = mybir.dt.bfloat16
AF = mybir.ActivationFunctionType

C = 1024      # embed dim
T = 2048      # sequence length
B = 4         # batch
NCOL = 512    # qkv columns per core (8 heads x 64)
TB = 512      # t-block / quarter size
SC = 128      # s-chunk size
D = 64        # head dim

N_WARM = 2   # PE clock-ramp dummy matmuls during startup DMA

LAST_RESULTS = None  # BassKernelResults of the most recent run (for test.py)
TRACE = False


def _build():
    N_PAIRS = NCOL // 128          # head-pairs per core (4)
    CC = C // 128                  # contraction chunks (8)
    N_TB = T // TB                 # t-blocks / quarters (4)
    SPB = TB // SC                 # s-chunks per t-block (4)
    N_TT = TB // SC                # t-subchunks per t-block (4)
    VGRP = 2 * N_PAIRS             # head groups in v_buf (8)
    VROW = VGRP * 65               # 520

    nc = bacc.Bacc("TRN2", target_bir_lowering=False, debug=False)

    xT = nc.dram_tensor("xT", (C, T), BF16, kind="ExternalInput")
    wq = nc.dram_tensor("wq", (C, NCOL), BF16, kind="ExternalInput")
    wk = nc.dram_tensor("wk", (C, NCOL), BF16, kind="ExternalInput")
    wv = nc.dram_tensor("wv", (C, NCOL), BF16, kind="ExternalInput")
    wp = nc.dram_tensor("wp", (NCOL, C), BF16, kind="ExternalInput")
    bq = nc.dram_tensor("bq", (NCOL, 1), F32, kind="ExternalInput")
    bk = nc.dram_tensor("bk", (NCOL, 1), F32, kind="ExternalInput")
    bv = nc.dram_tensor("bv", (1, NCOL), BF16, kind="ExternalInput")
    out = nc.dram_tensor("out", (T, C), BF16, kind="ExternalOutput")

    with tile.TileContext(nc) as tc, ExitStack() as ctx:
        const = ctx.enter_context(tc.tile_pool(name="const", bufs=1))
        xq_pool = ctx.enter_context(tc.tile_pool(name="xq", bufs=2))
        w_pool = ctx.enter_context(tc.tile_pool(name="wqkv", bufs=1))
        qt_pool = ctx.enter_context(tc.tile_pool(name="qt", bufs=2))
        att_pool = ctx.enter_context(tc.tile_pool(name="att", bufs=6))
        ynat_pool = ctx.enter_context(tc.tile_pool(name="ynat", bufs=2))
        yt_pool = ctx.enter_context(tc.tile_pool(name="yt", bufs=4))
        small = ctx.enter_context(tc.tile_pool(name="small", bufs=2))
        ostage = ctx.enter_context(tc.tile_pool(name="ostage", bufs=6))
        ps_acc = ctx.enter_context(tc.tile_pool(name="ps_acc", bufs=2, space="PSUM"))
        ps1 = ps_acc
        ps_po = ps_acc
        ps_sc = ctx.enter_context(tc.tile_pool(name="ps_sc", bufs=2, space="PSUM"))
        ps_yz = ctx.enter_context(tc.tile_pool(name="ps_yz", bufs=2, space="PSUM"))

        kT = const.tile([128, N_PAIRS * T], BF16, tag="kT")   # [col_in_pair, p*T + s]
        v_buf = const.tile([128, (T // SC) * VROW], BF16, tag="vbuf")
        wp_sb = const.tile([128, N_PAIRS * C], BF16, tag="wp")
        bq_sb = const.tile([128, N_PAIRS], F32, tag="bq")
        bk_sb = const.tile([128, N_PAIRS], F32, tag="bk")
        bv_sb = const.tile([1, NCOL], BF16, tag="bv")
        wq_sb = w_pool.tile([128, CC * NCOL], BF16, tag="wq")
        wk_sb = w_pool.tile([128, CC * NCOL], BF16, tag="wk")
        wv_sb = w_pool.tile([128, CC * NCOL], BF16, tag="wv")

        # startup DMAs: the v-units unblock first (xh+wv), then q, then k;
        # serial per-queue DMAs each run at full HBM bandwidth
        xh_tiles = {}
        xh_tiles[0] = xq_pool.tile([128, CC * TB], BF16, tag="xh", name="xh0")
        # All startup loads on ONE queue: the DMA device is serial and its
        # cross-queue arbitration is request-order FIFO, so a single queue is
        # the only way to get a deterministic priority order. x0 and wv are
        # streamed in interleaved cc-pair chunks so the first v-unit matmuls
        # (which consume chunk cc only) start ~1us in instead of waiting for
        # both full tensors (subtile deps gate each matmul on its own chunk).
        xsrc = xT.ap()[:, 0:TB].rearrange("(cc a) t -> a cc t", a=128)
        wvsrc = wv.ap().rearrange("(cc a) n -> a cc n", a=128)
        # chunk groups sized (1,1,2,4): the first matmuls start as soon as
        # the first 128-row chunk lands; later, larger chunks amortize the
        # ~600ns serialized per-DMA queue overhead
        cc0 = 0
        for n_cc in (2, 2, 2, 2):
            nc.sync.dma_start(
                xh_tiles[0][:, cc0 * TB : (cc0 + n_cc) * TB].rearrange(
                    "a (cc t) -> a cc t", cc=n_cc
                ),
                xsrc[:, cc0 : cc0 + n_cc],
            )
            nc.sync.dma_start(
                wv_sb[:, cc0 * NCOL : (cc0 + n_cc) * NCOL].rearrange(
                    "a (cc n) -> a cc n", cc=n_cc
                ),
                wvsrc[:, cc0 : cc0 + n_cc],
            )
            if cc0 == 0:
                nc.sync.dma_start(bv_sb[:], bv.ap())
            cc0 += n_cc
        # wq/wk in column halves (512B rows, still full DMA rate): pairs 0/1
        # only need the first half, so their attention starts ~3us earlier
        # while the second half streams in
        HC = NCOL // 2
        for half in range(2):
            nc.sync.dma_start(
                wq_sb[:].rearrange("a (cc n) -> a cc n", cc=CC)[
                    :, :, half * HC : (half + 1) * HC
                ],
                wq.ap().rearrange("(cc a) n -> a cc n", a=128)[
                    :, :, half * HC : (half + 1) * HC
                ],
            )
            if half == 0:
                nc.sync.dma_start(
                    bq_sb[:][:, :, None],
                    bq.ap().rearrange("(p a) o -> a p o", a=128),
                )
                nc.sync.dma_start(
                    bk_sb[:][:, :, None],
                    bk.ap().rearrange("(p a) o -> a p o", a=128),
                )
            nc.sync.dma_start(
                wk_sb[:].rearrange("a (cc n) -> a cc n", cc=CC)[
                    :, :, half * HC : (half + 1) * HC
                ],
                wk.ap().rearrange("(cc a) n -> a cc n", a=128)[
                    :, :, half * HC : (half + 1) * HC
                ],
            )
        # wp is not needed until the first output projection (~60us in)
        nc.sync.dma_start(
            wp_sb[:].rearrange("a (p n) -> a p n", p=N_PAIRS),
            wp.ap().rearrange("(p a) n -> a p n", a=128),
        )
        # 0/1 causal triangle mask: msk[s, f] = (f >= s); block-diagonal
        # offset r uses the width-(TB - r*SC) prefix of the same tile.
        # memset to 1.0 first (cheap, DMA-independent) so the PE warm-up can
        # start immediately; the triangle select rewrites it in place after.
        msk = const.tile([128, TB], BF16, tag="msk")
        nc.vector.memset(msk[:, 0:256], 1.0)
        # PE warm-up: dummy matmuls on the DMA-independent mask tile keep the
        # PE clock ramped while the input DMAs stream; a guard read into an
        # unused cell keeps them alive through DCE
        warm_ps = ps_sc.tile([128, 2 * TB], F32, tag="st", name="warm_ps")
        for _ in range(N_WARM):
            nc.tensor.matmul(
                warm_ps[:, 0:256], msk[:, 0:128], msk[:, 0:256], start=True, stop=True
            )
        nc.vector.memset(msk[:, 256:TB], 1.0)
        nc.gpsimd.affine_select(
            out=msk[:],
            in_=msk[:],
            compare_op=mybir.AluOpType.is_ge,
            fill=0.0,
            base=0,
            channel_multiplier=-1,
            pattern=[[1, TB]],
        )
        guard = const.tile([1, 1], BF16, tag="guard")
        nc.vector.tensor_copy(guard[:], warm_ps[0:1, 0:1])
        nc.sync.dma_start(out.ap()[0:1, 0:1], guard[:])
        ones_f32 = const.tile([128, max(128, (T // SC) * VGRP)], F32, tag="ones_f32")
        nc.vector.memset(ones_f32[:], 1.0)
        # bv broadcast across partitions once: the v-units then fold the bias
        # into their psum->SBUF copy instead of spending a PE matmul each
        bv_bc = const.tile([128, NCOL], BF16, tag="bv_bc")
        nc.gpsimd.partition_broadcast(bv_bc[:], bv_sb[:])
        # ones columns of v_buf (col 64 of each 65-group)
        nc.vector.tensor_copy(
            v_buf[:].rearrange("a (t g o) -> a t g o", g=VGRP, o=65)[:, :, :, 64:65],
            ones_f32[:, : (T // SC) * VGRP].rearrange("a (t g) -> a t g", g=VGRP)[
                :, :, :, None
            ],
        )

        # per-pair filler draw: sized to the exp-vs-PE deficit of each quarter
        # so earlier quarters don't starve the ACT-bound last quarter
        FILL_QUOTA = {
            0: [4, 3],
            1: [4, 3, 2, 2],
            2: [3, 2, 2, 1, 1, 1],
            3: [3, 2, 2, 2, 2, 2, 2, 1],
        }

        def head_units(tb, p, h):
            """Chunk-pair units of one attention head, for the global
            score/exp -> mask/AV software pipeline."""
            n_chunk = SPB * tb + SPB
            # diagonal chunks first: their exp->mask chain then overlaps with
            # the plain chunks' matmuls instead of stalling AV
            if tb > 0:
                j_order = (
                    [0, 1]
                    + list(range(SPB * tb, n_chunk))
                    + list(range(2, SPB * tb))
                )
            else:
                j_order = list(range(n_chunk))
            # per t-subchunk: positions in j_order of the first/last
            # contributing s-chunk (j contributes to tt iff j - SPB*tb <= tt)
            first_idx = {}
            last_idx = {}
            for idx, j in enumerate(j_order):
                r = j - SPB * tb
                for tt in range(max(r, 0), N_TT):
                    if tt not in first_idx:
                        first_idx[tt] = idx
                    last_idx[tt] = idx
            return [
                dict(
                    tb=tb, p=p, h=h, jj=jj, j_order=j_order, n_chunk=n_chunk,
                    first=first_idx, last=last_idx,
                )
                for jj in range(0, n_chunk, 2)
            ]

        def av_start_stop(u, idx, r, tt):
            """PSUM allows only ONE open accumulation group per 2KB bank
            (start marks the whole zero-region pending): the whole head's AV
            accumulation is a single group; the first write to each address
            overwrites, later writes accumulate."""
            r0 = u["j_order"][0] - SPB * u["tb"]
            start = idx == 0 and tt == max(r0, 0)
            stop = idx == u["n_chunk"] - 1 and tt == N_TT - 1
            return start, stop

        def score_exp(u):
            """Stage 1: score matmuls + exp for one chunk pair."""
            tb, p, h, jj = u["tb"], u["p"], u["h"], u["jj"]
            hrow = h * 64
            qT = qt_tiles[tb]
            st = ps_sc.tile([128, 2 * TB], F32, tag="st")
            at = att_pool.tile([128, 2 * TB], BF16, tag="at")
            # chunks are packed compactly (chunk k at offset o_k, width
            # TB - c0_k) so one exp instruction covers the whole pair
            cols = []
            o = 0
            for k in range(2):
                j = u["j_order"][jj + k]
                r = j - SPB * tb  # >=0 only for block-diag chunks
                c0 = max(0, r * SC)  # first valid t-col
                cols.append((jj + k, j, r, c0, o))
                nc.tensor.matmul(
                    st[:, o : o + TB - c0],
                    kT[hrow : hrow + 64, p * T + j * SC : p * T + j * SC + SC],
                    qT[hrow : hrow + 64, p * TB + c0 : (p + 1) * TB],
                    start=True,
                    stop=True,
                )
                o += TB - c0
            nc.scalar.activation(at[:, 0:o], st[:, 0:o], AF.Exp)
            u["at"] = at
            u["cols"] = cols

        head_yz = {}

        def mask_av(u):
            """Stage 2: causal mask + transposed-AV accumulation; emits the
            head's normalize after its last pair."""
            tb, p, h, jj = u["tb"], u["p"], u["h"], u["jj"]
            g = 2 * p + h
            at = u["at"]
            if jj == 0:
                head_yz[(tb, g)] = ps_yz.tile(
                    [128, N_TT * 65], F32, tag="yz", name=f"yz_{tb}_{g}"
                )
            yz = head_yz[(tb, g)]
            for idx, j, r, c0, o in u["cols"]:
                if r >= 0:
                    # zero att where t_loc < r*SC + s_loc
                    nc.vector.tensor_mul(
                        at[:, o : o + TB - c0],
                        at[:, o : o + TB - c0],
                        msk[:, 0 : TB - c0],
                    )
                vj = v_buf[:, j * VROW + g * 65 : j * VROW + g * 65 + 65]
                # transposed AV: att chunk stationary, [v | ones] moving;
                # accumulates yz[t, 0:64] = y and yz[t, 64] = Z
                for tt in range(max(r, 0), N_TT):
                    start, stop = av_start_stop(u, idx, r, tt)
                    nc.tensor.matmul(
                        yz[:, tt * 65 : tt * 65 + 65],
                        at[:, o + tt * SC - c0 : o + tt * SC - c0 + SC],
                        vj,
                        start=start,
                        stop=stop,
                    )
            if jj == u["n_chunk"] - 2:
                # normalize: y[t, d] / Z[t] for all 4 t-subchunks at once
                yzv = yz[:].rearrange("a (tt o) -> a tt o", o=65)
                rz = small.tile([128, N_TT], F32, tag="rz")
                nc.vector.reciprocal(rz[:][:, :, None], yzv[:, :, 64:65])
                ynat = ynat_tiles[tb]
                # pair-major ynat layout [t, (p, tt, 128)]: each pair's block
                # is contiguous, so ONE xbar DMA transposes it later
                nc.vector.tensor_mul(
                    ynat[:].rearrange(
                        "a (p tt c) -> a p tt c", p=N_PAIRS, tt=N_TT
                    )[:, p, :, h * 64 : h * 64 + 64],
                    yzv[:, :, 0:64],
                    rz[:][:, :, None].broadcast_to((128, N_TT, 64)),
                )
                head_yz.pop((tb, g))

        def emit_transpose(tb, p):
            """Transpose ynat[t, cols of pair p] -> yt[col, t] on the DMA
            xbar (out[c, tt, t] = in[t, tt*128+c]): one call per pair, 14ns
            per 16x128 tile on the ~17%-busy DMA device -- no PE matmuls, no
            DVE copies, no PSUM. Emitted a half-head after the normalize so
            the queue never parks on the input semaphore."""
            ynat = ynat_tiles[tb]
            yt = yt_tiles[tb]
            nc.sync.dma_start_transpose(
                yt[:, p * TB : (p + 1) * TB].rearrange(
                    "a (tt t) -> a tt t", tt=N_TT
                ),
                ynat[:, p * (N_TT * 128) : (p + 1) * (N_TT * 128)],
            )

        qt_tiles = {}
        ynat_tiles = {}
        yt_tiles = {}

        def qkv_thunks(tb):
            """Per-matmul thunks for quarter tb's projections, to be spliced
            one-at-a-time into the attention stream of quarter tb-1.
            Ordered v-first, then (q, k) per pair, so att(tb) head pairs can
            start as soon as their own pair's projections are consumed."""
            thunks = []
            t0 = tb * TB
            xh = xh_tiles[tb]
            v_units = []
            for tth in range(TB // 128):
                tt = (t0 // 128) + tth
                pt_box = [None]
                # quarter 0 runs its v-units chunk-major behind the startup
                # DMA stream; units 2/3 borrow idle score-pool banks so all
                # four accumulation groups can be open at once
                vpool, vtag = (
                    (ps_sc, "st") if tb == 0 and tth >= 2 else (ps1, "acc")
                )
                def mkv(cc, tth=tth, tt=tt, pt_box=pt_box, vpool=vpool, vtag=vtag):
                    def go():
                        if cc == 0:
                            pt_box[0] = vpool.tile([128, NCOL], F32, tag=vtag, name=f"psv_{tb}_{tth}")
                        pt = pt_box[0]
                        nc.tensor.matmul(
                            pt[:],
                            xh[:, cc * TB + tth * 128 : cc * TB + tth * 128 + 128],
                            wv_sb[:, cc * NCOL : (cc + 1) * NCOL],
                            start=(cc == 0),
                            stop=(cc == CC - 1),
                        )
                        if cc == CC - 1:
                            nc.vector.tensor_add(
                                v_buf[:, tt * VROW : (tt + 1) * VROW].rearrange(
                                    "a (g o) -> a g o", g=VGRP
                                )[:, :, 0:64],
                                pt[:].rearrange("a (g o) -> a g o", g=VGRP),
                                bv_bc[:].rearrange("a (g o) -> a g o", g=VGRP),
                            )
                    return go
                v_units.append([mkv(cc) for cc in range(CC)])
            if tb == 0:
                for cc in range(CC):
                    for un in v_units:
                        thunks.append(un[cc])
            else:
                for un in v_units:
                    thunks.extend(un)
            for u in range(2 * N_PAIRS):
                p, which = u // 2, u % 2
                wt, bias = ((wq_sb, bq_sb), (wk_sb, bk_sb))[which]
                dst = (
                    qt_tiles[tb][:, p * TB : (p + 1) * TB]
                    if which == 0
                    else kT[:, p * T + t0 : p * T + t0 + TB]
                )
                pt_box = [None]
                def mk(cc, u=u, p=p, wt=wt, bias=bias, dst=dst, pt_box=pt_box):
                    def go():
                        if cc == 0:
                            pt_box[0] = ps1.tile([128, TB], F32, tag="acc", name=f"ps_{tb}_{u}")
                        pt = pt_box[0]
                        nc.tensor.matmul(
                            pt[:],
                            wt[:, cc * NCOL + p * 128 : cc * NCOL + p * 128 + 128],
                            xh[:, cc * TB : cc * TB + TB],
                            start=(cc == 0),
                            stop=(cc == CC - 1),
                        )
                        if cc == CC - 1:
                            nc.vector.tensor_scalar_add(dst, pt[:], bias[:, p : p + 1])
                    return go
                thunks.extend(mk(cc) for cc in range(CC))
            return thunks

        def proj_thunks(tb, borrow=False):
            """Per-matmul thunks for t-block tb's output projection. With
            borrow=True (the final projection, after all attention), half the
            accumulator groups use the idle score pool's slots so four
            groups pipeline instead of two."""
            t0 = tb * TB
            yt = yt_tiles[tb]
            thunks = []
            for tt in range(TB // 128):
                for nh in range(C // 512):
                    gi = tt * (C // 512) + nh
                    po_box = [None]
                    def mk(p, tt=tt, nh=nh, gi=gi, po_box=po_box):
                        def go():
                            if p == 0:
                                if borrow and gi % 2 == 1:
                                    po_box[0] = ps_sc.tile(
                                        [128, 512], F32, tag="st",
                                        name=f"po_{tb}_{tt}_{nh}",
                                    )
                                else:
                                    po_box[0] = ps_po.tile(
                                        [128, 512], F32, tag="acc",
                                        name=f"po_{tb}_{tt}_{nh}",
                                    )
                            po = po_box[0]
                            nc.tensor.matmul(
                                po[:],
                                yt[:, p * TB + tt * 128 : p * TB + tt * 128 + 128],
                                wp_sb[:, p * C + nh * 512 : p * C + nh * 512 + 512],
                                start=(p == 0),
                                stop=(p == N_PAIRS - 1),
                            )
                            if p == N_PAIRS - 1:
                                ob = ostage.tile([128, 512], BF16, tag="ob")
                                nc.vector.tensor_copy(ob[:], po[:])
                                dq = nc.scalar if borrow and gi % 2 else nc.sync
                                dq.dma_start(
                                    out.ap()[
                                        t0 + tt * 128 : t0 + tt * 128 + 128,
                                        nh * 512 : (nh + 1) * 512,
                                    ],
                                    ob[:],
                                )
                        return go
                    thunks.extend(mk(p) for p in range(N_PAIRS))
            return thunks

        # ---- global schedule ----
        # Two filler queues, consumed by fill() calls placed right after each
        # exp emission (the PE wait queue releases IN ORDER, so filler behind
        # a parked AV matmul cannot run early -- it must precede the AVs):
        #   gated:   qkv projection thunks + transposes; their execution also
        #            gates attention heads (per-pair markers).
        #   reserve: output-projection thunks, saved for the ACT-bound tail
        #            (the last quarter has the most exp work and the least
        #            attention-independent PE work).
        gated = []
        reserve = []
        gpos = [0]
        rpos = [0]

        def fill(k):
            take = min(k, len(gated) - gpos[0])
            for th in gated[gpos[0] : gpos[0] + take]:
                th()
            gpos[0] += take
            k -= take
            take = min(k, len(reserve) - rpos[0])
            for th in reserve[rpos[0] : rpos[0] + take]:
                th()
            rpos[0] += take

        def gate(idx):
            while gpos[0] < idx:
                gated[gpos[0]]()
                gpos[0] += 1

        pair_marker = {}

        def stage_qkv(tb, issue_dma=True):
            """Issue x prefetch + append quarter tb's projection thunks."""
            t0 = tb * TB
            if issue_dma:
                nxt = xq_pool.tile([128, CC * TB], BF16, tag="xh", name=f"xh{tb}")
                xh_tiles[tb] = nxt
                nc.sync.dma_start(
                    nxt[:].rearrange("a (cc t) -> a cc t", cc=CC),
                    xT.ap()[:, t0 : t0 + TB].rearrange("(cc a) t -> a cc t", a=128),
                )
            qt_tiles[tb] = qt_pool.tile(
                [128, N_PAIRS * TB], BF16, tag="qT", name=f"qT{tb}"
            )
            base = len(gated)
            gated.extend(qkv_thunks(tb))
            n_v = (TB // 128) * CC
            for p in range(N_PAIRS):
                pair_marker[(tb, p)] = base + n_v + 2 * CC * (p + 1)

        # The unit stream: per head, chunk-pair units; stage-1 (score+exp) of
        # unit i+1 is emitted BEFORE stage-2 (mask+AV) of unit i, so the exp
        # of the next pair runs on ScalarE while the PE processes the current
        # pair's AVs -- without this the in-order PE queue serializes
        # exp -> AV -> next scores -> next exp.
        stage_qkv(0, issue_dma=False)

        units = []
        for tbx in range(N_TB):
            for p in range(N_PAIRS):
                for h in range(2):
                    units.extend(head_units(tbx, p, h))

        started = set()
        pending_tp = []
        prev = None
        for u in units:
            tbx, p, h = u["tb"], u["p"], u["h"]
            if u["jj"] == 0 and h == 0 and p == 0 and tbx not in started:
                started.add(tbx)
                if tbx + 1 < N_TB:
                    stage_qkv(tbx + 1)
                xh_tiles.pop(tbx, None)
                ynat_tiles[tbx] = ynat_pool.tile(
                    [128, N_TT * NCOL], BF16, tag="ynat", name=f"ynat{tbx}"
                )
                yt_tiles[tbx] = yt_pool.tile(
                    [128, N_PAIRS * TB], BF16, tag="yt", name=f"yt{tbx}"
                )
            if u["jj"] == 0:
                # pair p's q/k/v thunks must execute before its scores
                gate(pair_marker[(tbx, p)])
            score_exp(u)
            if prev is not None:
                fill(FILL_QUOTA[prev["tb"]][prev["jj"] // 2])
                mask_av(prev)
                if prev["jj"] == prev["n_chunk"] - 2:
                    # head boundary: flush one pending transpose (its DVE
                    # normalize dependency is a full head old by now)
                    if pending_tp:
                        emit_transpose(*pending_tp.pop(0))
                    if prev["h"] == 1:
                        pending_tp.append((prev["tb"], prev["p"]))
                        if prev["p"] == N_PAIRS - 1 and prev["tb"] < N_TB - 1:
                            reserve.extend(proj_thunks(prev["tb"]))
            prev = u
        fill(FILL_QUOTA[prev["tb"]][prev["jj"] // 2])
        mask_av(prev)
        pending_tp.append((prev["tb"], prev["p"]))

        # drain remaining transposes and filler, then the final projection
        for tp_args in pending_tp:
            emit_transpose(*tp_args)
        gate(len(gated))
        fill(len(reserve) - rpos[0])
        for th in proj_thunks(N_TB - 1, borrow=True):
            th()

    nc.compile()
    return nc


_NC_CACHE = None


def kernel(x, Wq, bq, Wk, bk, Wv, bv, Wp, bp):
    global LAST_RESULTS, _NC_CACHE
    import ml_dtypes

    bf16 = ml_dtypes.bfloat16
    x = np.asarray(x, dtype=np.float32)
    Wq = np.asarray(Wq, dtype=np.float32)
    Wk = np.asarray(Wk, dtype=np.float32)
    Wv = np.asarray(Wv, dtype=np.float32)
    Wp = np.asarray(Wp, dtype=np.float32)
    bq = np.asarray(bq, dtype=np.float32)
    bk = np.asarray(bk, dtype=np.float32)
    bv = np.asarray(bv, dtype=np.float32)
    bp = np.asarray(bp, dtype=np.float32)

    if _NC_CACHE is None:
        _NC_CACHE = _build()
    nc = _NC_CACHE

    scale = 1.0 / np.sqrt(D)
    # cores 2b and 2b+1 share x[b].T; cores with the same head-group share
    # the weight slices -- compute each unique tensor once
    xts = [np.ascontiguousarray(x[b].T).astype(bf16) for b in range(B)]
    wsets = []
    for hg in range(2):
        cols = slice(hg * NCOL, (hg + 1) * NCOL)
        wsets.append(
            {
                "wq": (np.ascontiguousarray(Wq[:, cols]) * scale).astype(bf16),
                "wk": np.ascontiguousarray(Wk[:, cols]).astype(bf16),
                "wv": np.ascontiguousarray(Wv[:, cols]).astype(bf16),
                "wp": np.ascontiguousarray(Wp[cols, :]).astype(bf16),
                "bq": (bq[cols] * scale).reshape(NCOL, 1).copy(),
                "bk": bk[cols].reshape(NCOL, 1).copy(),
                "bv": bv[cols].reshape(1, NCOL).astype(bf16),
            }
        )
    in_maps = [
        {"xT": xts[core // 2], **wsets[core % 2]} for core in range(8)
    ]

    res = run_bass_kernel_spmd(nc, in_maps, core_ids=list(range(8)), trace=TRACE)
    LAST_RESULTS = res

    result = np.empty((B, T, C), dtype=np.float32)
    for b in range(B):
        result[b] = (
            res.results[2 * b]["out"].astype(np.float32)
            + res.results[2 * b + 1]["out"].astype(np.float32)
            + bp
        )
    return result
